# revision 35
# baseline (speedup 1.0000x reference)
"""Trainium2 Bass kernel for the DCGSC SNN (delayed-current adaptive-LIF net).

Math (per layer, after folding BN + (1-alpha) into the weights):
    v_t = alpha * w_{t-1} + Itil_t - y_{t-1}
    w_t = v_t * 1[v_t <= TH]          (soft-reset membrane)
    s_t = 1[v_t > TH]                 (spike)
    y_t = rho * y_{t-1} + gamma * s_t (gamma = (1-alpha)*beta_a; y = (1-alpha)*a)
Itil_t comes from a PSUM matmul accumulation; the -y_{t-1} term is folded into
the same PSUM bank via two identity matmuls of the expanded adaptation:
    -y_{t-1} = (-rho*I) @ Y_{t-2} + (-I) @ sigma_{t-1},   sigma = gamma*s,
so the only chain-critical PE op after sigma_{t-1} is one FD=256 matmul.
Per-channel learnable delays are applied as time-offset DMA reads: layer-1
from a host-transposed zero-padded input, layer-2 via a DRAM spike staging
buffer read back with per-delay-group offsets (channels host-sorted by delay).
Output layer: mean_t(leaky v_out) == Wout @ (sum_t c_t * s2_t), accumulated
per-step in PSUM with c_t-scaled spikes.

The spike dynamics are chaotically sensitive (near-threshold flips cascade),
so every term feeding the membrane must be fp32-exact. Feedforward matmuls
use a scaled relative fp16 split: moving tensors (0/1 spikes, ones) carry a
global 2^-11 factor (exact in fp16), weights are hi*2^11 (exact exponent
shift) plus lo' = (W - hi)*2^11 — two fp16 matmuls reconstruct W@x to ~22
mantissa bits at 1 PE cycle/row. The adaptation state Y' = y/(gamma*2^11)
is updated directly from the staged spikes; its -rho*y matmul stays fp32.
Membrane v/w and Y' stay fp32 on the Vector engine.

Sharding: pure data-parallel, batch 512 -> 64 per core across 8 cores.
"""

import sys

sys.path.insert(0, "/opt/trn_rl_repo")

import numpy as np

B, T, FIN, H, C = 512, 100, 140, 512, 35
MAX_DELAY = 60
TH = 0.3
EPS = 1e-5
NCORES = 8
BL = B // NCORES  # 64
TB = 4  # time-block: one [128, 2*TB*128] PSUM tile (2 banks) per block
TPAD = T + MAX_DELAY


def _sigmoid32(x):
    x = np.asarray(x, np.float32)
    return (1.0 / (1.0 + np.exp(-x.astype(np.float64)))).astype(np.float32)


def _delays(delay_raw):
    return np.round(_sigmoid32(delay_raw) * np.float32(MAX_DELAY)).astype(np.int64)


def _groups(ds):
    """Runs of equal delay in sorted order, split at 128-partition chunks.
    Returns list of (chunk, p0, p1, delay)."""
    out = []
    i = 0
    n = len(ds)
    while i < n:
        j = i
        while j < n and ds[j] == ds[i]:
            j += 1
        s = i
        while s < j:
            e = min(j, (s // 128 + 1) * 128)
            out.append((s // 128, s % 128, (e - 1) % 128 + 1, int(ds[i])))
            s = e
        i = j
    return out


def _numpy_reference(i):
    """Fallback path (only used if params are not per-channel-constant)."""
    x = i["x"]

    def ad(x, draw):
        d = _delays(draw)
        Bb, Tt, Ff = x.shape
        xp = np.pad(x, ((0, 0), (MAX_DELAY, 0), (0, 0)))
        idx = np.arange(Tt)[:, None] + MAX_DELAY - d[None, :]
        return np.take_along_axis(xp, np.broadcast_to(idx[None], (Bb, Tt, Ff)), axis=1)

    def bn(v, g, b, m, s):
        return (v - m) / np.sqrt(s + EPS) * g + b

    def adlif(I, al, rh, ba):
        v = np.zeros(I.shape[1:], np.float32)
        a = np.zeros_like(v)
        s = np.zeros_like(v)
        out = []
        for t in range(I.shape[0]):
            v = al * v * (1 - s) + (1 - al) * (I[t] - a)
            s = (v > TH).astype(np.float32)
            a = rh * a + ba * s
            out.append(s)
        return np.stack(out)

    xd = ad(x, i["delay_raw1"])
    I1 = bn(np.einsum("btf,hf->bth", xd, i["W1"]), i["gamma1"], i["bias1"], i["mean1"], i["var1"])
    s1 = adlif(np.transpose(I1, (1, 0, 2)), i["alpha1"], i["rho1"], i["beta_a1"])
    sd = ad(np.transpose(s1, (1, 0, 2)), i["delay_raw2"])
    I2 = bn(np.einsum("bth,gh->btg", sd, i["W2"]), i["gamma2"], i["bias2"], i["mean2"], i["var2"])
    s2 = adlif(np.transpose(I2, (1, 0, 2)), i["alpha2"], i["rho2"], i["beta_a2"])
    Io = np.einsum("tbh,ch->tbc", s2, i["Wout"])
    v = np.zeros(Io.shape[1:], np.float32)
    acc = np.zeros_like(v)
    for t in range(T):
        v = i["beta_out"] * v + (1 - i["beta_out"]) * Io[t]
        acc += v
    return (acc / T).astype(np.float32)


def _build_program(d1groups, d2groups, sc):
    """Emit the SPMD Bass/Tile program (identical across cores)."""
    import concourse.bacc as bacc
    import concourse.mybir as mybir
    import concourse.tile as tile
    from contextlib import ExitStack

    f32 = mybir.dt.float32
    bf16 = mybir.dt.float16  # 11-bit mantissa: split hi+lo covers 22+ bits
    AL = mybir.AluOpType

    def R(ap):
        return ap

    nc = bacc.Bacc(
        "TRN2",
        target_bir_lowering=False,
        debug=False,
        enable_asserts=False,
        num_devices=NCORES,
    )

    xt_d = nc.dram_tensor("xt", [FIN, T, BL], bf16, kind="ExternalInput")
    w1ah_d = nc.dram_tensor("w1ah", [128, H], bf16, kind="ExternalInput")
    w1al_d = nc.dram_tensor("w1al", [128, H], bf16, kind="ExternalInput")
    w1b_d = nc.dram_tensor("w1b", [26, H], bf16, kind="ExternalInput")
    w2h_d = nc.dram_tensor("w2h", [H, H], bf16, kind="ExternalInput")
    w2l_d = nc.dram_tensor("w2l", [H, H], bf16, kind="ExternalInput")
    c2_d = nc.dram_tensor("c2", [2, H], bf16, kind="ExternalInput")
    wo_d = nc.dram_tensor("wo", [H, C], f32, kind="ExternalInput")
    ones_d = nc.dram_tensor("onesr", [2, T * BL], bf16, kind="ExternalInput")
    idn_d = nc.dram_tensor("idn", [128, 128], bf16, kind="ExternalInput")
    ngh1_d = nc.dram_tensor("ngh1", [128, 128], bf16, kind="ExternalInput")
    ngl1_d = nc.dram_tensor("ngl1", [128, 128], bf16, kind="ExternalInput")
    ngh2_d = nc.dram_tensor("ngh2", [128, 128], bf16, kind="ExternalInput")
    ngl2_d = nc.dram_tensor("ngl2", [128, 128], bf16, kind="ExternalInput")
    nr1_d = nc.dram_tensor("nr1", [128, 128], f32, kind="ExternalInput")
    nr2_d = nc.dram_tensor("nr2", [128, 128], f32, kind="ExternalInput")
    out_d = nc.dram_tensor("out", [BL, C], f32, kind="ExternalOutput")
    import os as _os
    _dbg = _os.environ.get("BASSDBG") == "1"
    sdram = nc.dram_tensor("sdram", [H, T, BL], bf16,
                           kind="ExternalOutput" if _dbg else "Internal")

    with tile.TileContext(nc) as tc:
        with ExitStack() as ctx:
            P = ctx.enter_context(tc.tile_pool(name="persist", bufs=1))
            PS = ctx.enter_context(tc.tile_pool(name="pstep", bufs=3, space="PSUM"))
            PX = ctx.enter_context(tc.tile_pool(name="psmall", bufs=1, space="PSUM"))
            VP = ctx.enter_context(tc.tile_pool(name="vp", bufs=8))
            SP = ctx.enter_context(tc.tile_pool(name="sp", bufs=12))

            xda = P.tile([128, T, BL], bf16, name="xda", tag="xda")
            xdb = P.tile([26, T, BL], bf16, name="xdb", tag="xdb")
            sdt = [P.tile([128, T, BL], bf16, name=f"sdt{k}", tag=f"sdt{k}") for k in range(4)]
            w1ah = P.tile([128, H], bf16, name="w1ah", tag="w1ah")
            w1al = P.tile([128, H], bf16, name="w1al", tag="w1al")
            w1b = P.tile([26, H], bf16, name="w1b", tag="w1b")
            w2h = [P.tile([128, H], bf16, name=f"w2h{k}", tag=f"w2h{k}") for k in range(4)]
            w2l = [P.tile([128, H], bf16, name=f"w2l{k}", tag=f"w2l{k}") for k in range(4)]
            c2r = P.tile([2, H], bf16, name="c2r", tag="c2r")
            wo = [P.tile([128, C], f32, name=f"wo{k}", tag=f"wo{k}") for k in range(4)]
            idn = P.tile([128, 128], bf16, name="idn", tag="idn")
            ngh = [P.tile([128, 128], bf16, name=f"ngh{l}", tag=f"ngh{l}") for l in range(2)]
            ngl = [P.tile([128, 128], bf16, name=f"ngl{l}", tag=f"ngl{l}") for l in range(2)]
            nr1 = P.tile([128, 128], f32, name="nr1", tag="nr1")
            nr2 = P.tile([128, 128], f32, name="nr2", tag="nr2")
            w1t = P.tile([128, 256], f32, name="w1t", tag="w1t")
            w2t = P.tile([128, 256], f32, name="w2t", tag="w2t")
            Y1 = [P.tile([128, 256], f32, name=f"Y1{k}", tag=f"Y1{k}") for k in range(2)]
            Y2 = [P.tile([128, 256], f32, name=f"Y2{k}", tag=f"Y2{k}") for k in range(2)]
            ones1 = P.tile([2, TB * BL], bf16, name="ones1", tag="ones1")
            Ssb = P.tile([128, 256], f32, name="Ssb", tag="Ssb")
            osb = P.tile([BL, C], f32, name="osb", tag="osb")

            # weight loads
            nc.scalar.dma_start(w1ah[:], w1ah_d.ap())
            nc.scalar.dma_start(w1al[:], w1al_d.ap())
            nc.scalar.dma_start(w1b[:], w1b_d.ap())
            nc.scalar.dma_start(ngh[0][:], ngh1_d.ap())
            nc.scalar.dma_start(ngl[0][:], ngl1_d.ap())
            nc.scalar.dma_start(nr1[:], nr1_d.ap())

            def load_l2_weights():
                w2hr = w2h_d.ap().rearrange("(k p) h -> k p h", p=128)
                w2lr = w2l_d.ap().rearrange("(k p) h -> k p h", p=128)
                for k in range(4):
                    nc.scalar.dma_start(w2h[k][:], w2hr[k])
                    nc.sync.dma_start(w2l[k][:], w2lr[k])
                nc.sync.dma_start(c2r[:], c2_d.ap())
                wor = wo_d.ap().rearrange("(k p) c -> k p c", p=128)
                for k in range(4):
                    nc.scalar.dma_start(wo[k][:], wor[k])
                nc.sync.dma_start(idn[:], idn_d.ap())
                nc.scalar.dma_start(ngh[1][:], ngh2_d.ap())
                nc.scalar.dma_start(ngl[1][:], ngl2_d.ap())
                nc.sync.dma_start(nr2[:], nr2_d.ap())

            # zero-init (layer-2 delayed-spike pads + scan state)
            for k in range(4):
                nc.gpsimd.memset(sdt[k][:], 0.0)
            for tl in (w1t, w2t):
                nc.vector.memset(tl[:], 0.0)
            for tl in (Y1[0], Y1[1], Y2[0], Y2[1]):
                nc.vector.memset(tl[:], 0.0)
            nc.vector.memset(ones1[:], 1.0 / 2048.0)

            # layer-1 delays are applied host-side; xt is already shifted and
            # zero-padded, so the input loads are dense full-partition DMAs.
            # Tail channels (128:140) load twice: rows 0-11 for the hi
            # weights, rows 12-23 for the lo weights.
            _dmae = [nc.sync, nc.scalar]
            nc.sync.dma_start(xda[:, :, :], xt_d.ap()[0:128, :, :])
            nc.sync.dma_start(xdb[0:12, :, :], xt_d.ap()[128:140, :, :])
            nc.sync.dma_start(xdb[12:24, :, :], xt_d.ap()[128:140, :, :])
            # bias rows (ones, hi+lo)
            nc.sync.dma_start(xdb[24:26].rearrange("p t b -> p (t b)"), ones_d.ap())

            sdram_r = sdram.ap().rearrange("(k p) t b -> k p t b", p=128)

            stage_at = {}

            def scan(layer, after_block=None):
                wt = w1t if layer == 1 else w2t
                Ys = Y1 if layer == 1 else Y2
                nrho = nr1 if layer == 1 else nr2
                alpha = sc["a1"] if layer == 1 else sc["a2"]
                rho = sc["r1"] if layer == 1 else sc["r2"]
                th = sc["th1"] if layer == 1 else sc["th2"]
                nh, nl = ngh[layer - 1], ngl[layer - 1]
                sig_prev = None
                psS = None
                if layer == 2:
                    psS = PX.tile([128, 256], f32, name="psS", tag="psS")
                for blk in range(T // TB):
                    t0 = blk * TB
                    ps = PS.tile([128, 2 * TB * 128], f32, name="ps", tag="ps")
                    # column map: bq*TB*128 + t*128 + qi*64 + b  (q = 2*bq+qi)
                    psv = ps[:].rearrange("p (bq t qi b) -> p bq t qi b",
                                          bq=2, t=TB, qi=2)
                    # NOTE: start=True clears has_written for the WHOLE 2KB
                    # bank, so only the first matmul into each bank carries it.
                    for q in range(4):
                        bq, qi = q // 2, q % 2
                        dst = psv[:, bq, :, qi, :]
                        if layer == 1:
                            nc.tensor.matmul(
                                dst, w1ah[:, q * 128 : (q + 1) * 128],
                                xda[:, t0 : t0 + TB, :],
                                start=(qi == 0), stop=False, skip_group_check=True)
                            nc.tensor.matmul(
                                dst, w1al[:, q * 128 : (q + 1) * 128],
                                xda[:, t0 : t0 + TB, :],
                                start=False, stop=False, skip_group_check=True)
                            nc.tensor.matmul(
                                dst, w1b[:, q * 128 : (q + 1) * 128],
                                xdb[:, t0 : t0 + TB, :],
                                start=False, stop=False, skip_group_check=True)
                        else:
                            for k in range(4):
                                nc.tensor.matmul(
                                    dst, w2h[k][:, q * 128 : (q + 1) * 128],
                                    sdt[k][:, t0 : t0 + TB, :],
                                    start=(qi == 0 and k == 0), stop=False,
                                    skip_group_check=True)
                                nc.tensor.matmul(
                                    dst, w2l[k][:, q * 128 : (q + 1) * 128],
                                    sdt[k][:, t0 : t0 + TB, :],
                                    start=False, stop=False, skip_group_check=True)
                            nc.tensor.matmul(
                                dst, c2r[:, q * 128 : (q + 1) * 128], ones1[:],
                                start=False, stop=False, skip_group_check=True)
                    for tt in range(TB):
                        t = t0 + tt
                        # -y_{t-1} = (-rho I) @ Y_{t-2}  +  (-I) @ sigma_{t-1}
                        for bq in range(2):
                            pb = ps[:, bq * TB * 128 + tt * 128 :
                                       bq * TB * 128 + (tt + 1) * 128]
                            if t >= 2:
                                nc.tensor.matmul(
                                    pb, nrho[:], Ys[t % 2][:, bq * 128 : (bq + 1) * 128],
                                    start=False, stop=False, skip_group_check=True)
                            if t >= 1:
                                nc.tensor.matmul(
                                    pb, nh[:], sig_prev[:, bq * 128 : (bq + 1) * 128],
                                    start=False, stop=False, skip_group_check=True)
                                nc.tensor.matmul(
                                    pb, nl[:], sig_prev[:, bq * 128 : (bq + 1) * 128],
                                    start=False, stop=(tt == TB - 1),
                                    skip_group_check=True)
                        pst = psv[:, :, tt, :, :]
                        v = VP.tile([128, 256], f32, name="v", tag="v")
                        vv = v[:].rearrange("p (bq qi b) -> p bq qi b", bq=2, qi=2)
                        wv = wt[:].rearrange("p (bq qi b) -> p bq qi b", bq=2, qi=2)
                        nc.vector.scalar_tensor_tensor(
                            vv, wv, alpha, pst, op0=AL.mult, op1=AL.add)
                        s01 = SP.tile([128, 256], bf16, name=f"s01{layer}", tag=f"s01{layer}")
                        nc.vector.tensor_scalar(
                            s01[:], v[:], th, float(sc["s01"]),
                            op0=AL.is_gt, op1=AL.mult)
                        nc.vector.scalar_tensor_tensor(
                            wt[:], v[:], th, v[:], op0=AL.is_le, op1=AL.mult)
                        if layer == 2:
                            sh = SP.tile([128, 256], bf16, name="sh", tag="sh")
                            nc.vector.tensor_scalar(
                                sh[:], s01[:], float(sc["ct"][t] * 2048.0), None,
                                op0=AL.mult, op1=AL.bypass)
                            nc.tensor.matmul(
                                psS[:], idn[:], sh[:], start=(t == 0), stop=(t == T - 1),
                                skip_group_check=True)
                        # y_t = rho*y_{t-1} + sigma_t (one step of slack before
                        # its consumer, the t+2 nrho matmul)
                        nc.vector.scalar_tensor_tensor(
                            Ys[t % 2][:], Ys[(t + 1) % 2][:], rho, s01[:],
                            op0=AL.mult, op1=AL.add)
                        if layer == 1:
                            stage_at[t] = nc.sync.dma_start(
                                sdram.ap()[:, t, :].rearrange("(k p) b -> p k b", p=128),
                                s01[:].rearrange("p (k b) -> p k b", b=64))
                        sig_prev = s01
                    if after_block and blk in after_block and tt == TB - 1:
                        after_block[blk]()
                return psS

            from concourse.bass import _add_dep_helper

            NCH = 5
            CH = T // NCH

            def emit_d2_chunk(j):
                # layer-2 delayed spikes: sdt[k][p,t,b] = sig1[k*128+p, t-d, b].
                # Emitted mid-L1-scan, gated on the staging DMA of the last
                # needed step via explicit dep edges (no barrier); qACT only,
                # so the reads never queue ahead of qSP staging writes.
                ta, tb_ = j * CH, (j + 1) * CH
                gate = stage_at[tb_ - 1]
                for ch, p0, p1, d in d2groups:
                    lo = max(ta, d)
                    if lo < tb_:
                        inst = nc.scalar.dma_start(
                            sdt[ch][p0:p1, lo:tb_, :],
                            sdram_r[ch, p0:p1, lo - d : tb_ - d, :])
                        _add_dep_helper(
                            getattr(inst, "ins", inst),
                            getattr(gate, "ins", gate),
                            sync=True, reason="sdram staging RAW")

            cbs = {0: load_l2_weights}
            for j in range(NCH):
                cbs[(CH * (j + 1) - 1) // TB] = (lambda jj: lambda: emit_d2_chunk(jj))(j)

            scan(1, after_block=cbs)

            psS = scan(2)

            # output: out[b, c] = sum_q S[q-chunk]^T @ WoutT[q-chunk]
            nc.vector.tensor_copy(Ssb[:], psS[:])
            psO = PX.tile([BL, C], f32, name="psO", tag="psO")
            for q in range(4):
                nc.tensor.matmul(
                    psO[:], Ssb[:, q * 64 : (q + 1) * 64], wo[q][:],
                    start=(q == 0), stop=(q == 3), skip_group_check=True)
            nc.vector.tensor_copy(osb[:], psO[:])
            nc.sync.dma_start(out_d.ap(), osb[:])

    nc.compile()
    return nc


_CACHE = {}


def _prep_and_run(inputs, trace=False):
    i = {k: np.asarray(v, np.float32) for k, v in inputs.items()}
    const = all(
        np.ptp(np.asarray(i[k], np.float64)) == 0.0
        for k in ("alpha1", "rho1", "beta_a1", "alpha2", "rho2", "beta_a2", "beta_out")
    )
    if not const or i["x"].shape != (B, T, FIN):
        return _numpy_reference(i), None

    d1 = _delays(i["delay_raw1"])
    d2 = _delays(i["delay_raw2"])
    fperm = np.argsort(d1, kind="stable")
    hperm = np.argsort(d2, kind="stable")
    d1groups = _groups(d1[fperm])
    d2groups = _groups(d2[hperm])

    a1 = float(i["alpha1"][0]); r1 = float(i["rho1"][0]); b1 = float(i["beta_a1"][0])
    a2 = float(i["alpha2"][0]); r2 = float(i["rho2"][0]); b2 = float(i["beta_a2"][0])
    bo = float(i["beta_out"][0])
    ga1 = np.float32((1 - a1) * b1)
    ga2 = np.float32((1 - a2) * b2)

    bf = np.float16

    # no rescale: sigma stays fp32 (exact gamma); layer-2 input spikes are
    # staged raw as 0/1 (exact in fp16), with gamma folded into nothing.
    gt1 = float(ga1)
    gt2 = float(ga2)
    th1 = TH
    th2 = TH

    S11 = np.float64(2048.0)  # moving tensors carry 2^-11; weights carry 2^11

    def split_bf(a):
        # hi*2^11 is an exact fp16 exponent shift; lo' = (a - hi)*2^11 recovers
        # ~11 more mantissa bits. Paired with 2^-11-scaled spikes the products
        # reconstruct a*s to ~2^-22 relative - fp32-class, flip-free.
        a64 = np.asarray(a, np.float64)
        hi = np.asarray(a, np.float32).astype(bf)
        hi2 = (hi.astype(np.float32) * np.float32(S11)).astype(bf)
        lo2 = ((a64 - hi.astype(np.float64)) * S11).astype(np.float32).astype(bf)
        return hi2, lo2

    g1 = (i["gamma1"].astype(np.float64) / np.sqrt(i["var1"].astype(np.float64) + EPS))
    W1f = (i["W1"].astype(np.float64) * g1[:, None] * (1 - a1))[hperm][:, fperm]
    c1f = ((i["bias1"].astype(np.float64) - i["mean1"].astype(np.float64) * g1)[hperm]
           * (1 - a1))
    g2 = (i["gamma2"].astype(np.float64) / np.sqrt(i["var2"].astype(np.float64) + EPS))
    W2f = (i["W2"].astype(np.float64) * g2[:, None] * (1 - a2))[:, hperm]
    c2f = (i["bias2"].astype(np.float64) - i["mean2"].astype(np.float64) * g2) * (1 - a2)

    w1ah, w1al = split_bf(np.ascontiguousarray(W1f[:, :128].T))      # [128, 512]
    w1b = np.zeros((26, H), np.float64)
    tl_hi, tl_lo = split_bf(W1f[:, 128:].T)                          # [12, 512]
    c1_hi, c1_lo = split_bf(c1f)
    w1b = np.zeros((26, H), bf)
    w1b[:12] = tl_hi; w1b[12:24] = tl_lo; w1b[24] = c1_hi; w1b[25] = c1_lo
    w2h, w2l = split_bf(np.ascontiguousarray(W2f.T))                 # [512h1, 512g]
    c2_hi, c2_lo = split_bf(c2f)
    c2r = np.stack([c2_hi, c2_lo])                                   # [2, 512]
    woT = np.ascontiguousarray(i["Wout"].T, np.float32)              # [512, 35]
    ct = ((1.0 - bo ** (T - np.arange(T))) / T).astype(np.float32)

    eye = np.eye(128, dtype=np.float32)
    # -y matmul weights: A = gamma*2^11 paired with s*2^-11 spikes; hi/lo
    # split recovers gamma to ~22 bits. Y' tracks y/(gamma*2^11) so its
    # update input is the raw s*2^-11 tensor.
    def gsplit(g):
        A = np.float64(g) * 2048.0
        hi = np.float16(A)
        lo = np.float16(np.float32(A - np.float64(hi)))
        return float(hi), float(lo)
    g1h, g1l = gsplit(ga1)
    g2h, g2l = gsplit(ga2)
    sc = dict(a1=a1, r1=r1, a2=a2, r2=r2, gt1=gt1, gt2=gt2,
              th1=th1, th2=th2, ct=ct, s01=1.0 / 2048.0)

    key = (tuple(d1groups), tuple(d2groups),
           a1, r1, b1, a2, r2, b2, bo)
    if key not in _CACHE:
        _CACHE[key] = _build_program(d1groups, d2groups, sc)
    nc = _CACHE[key]

    # host-applied layer-1 delays: xt[f, t, b] = x[f, t - d_f, b] (zero pad),
    # at the 2^-11 moving-tensor scale
    xsrc = (i["x"].transpose(2, 1, 0)[fperm] * np.float32(1.0 / 2048.0)).astype(bf)
    xt_full = np.zeros((FIN, T, B), bf)
    ds = d1[fperm]
    for f in range(FIN):
        d = int(ds[f])
        if d < T:
            xt_full[f, d:T] = xsrc[f, 0 : T - d]
    shared = dict(w1ah=w1ah, w1al=w1al, w1b=w1b, w2h=w2h, w2l=w2l, c2=c2r, wo=woT,
                  onesr=np.full((2, T * BL), 1.0 / 2048.0, bf),
                  idn=eye.astype(bf),
                  ngh1=(-np.float32(g1h) * eye).astype(bf),
                  ngl1=(-np.float32(g1l) * eye).astype(bf),
                  ngh2=(-np.float32(g2h) * eye).astype(bf),
                  ngl2=(-np.float32(g2l) * eye).astype(bf),
                  nr1=np.float32(-r1 * float(ga1) * 2048.0) * eye,
                  nr2=np.float32(-r2 * float(ga2) * 2048.0) * eye)
    in_maps = []
    for c in range(NCORES):
        m = dict(shared)
        m["xt"] = np.ascontiguousarray(xt_full[:, :, c * BL : (c + 1) * BL])
        in_maps.append(m)

    from concourse.bass_utils import run_bass_kernel_spmd

    res = run_bass_kernel_spmd(nc, in_maps, list(range(NCORES)), trace=trace)
    out = np.concatenate([res.results[c]["out"] for c in range(NCORES)], axis=0)
    return out.astype(np.float32), res


def kernel(**inputs):
    out, _ = _prep_and_run(inputs, trace=False)
    return out


def _install_ntff_hook():
    """Provide antenv.axon_hooks (missing in this image) so trace=True works."""
    import types, ctypes, contextlib

    try:
        import antenv.axon_hooks  # noqa: F401
        return
    except ImportError:
        pass
    so_path = "/opt/axon/libaxon_pjrt.so"
    hook = None
    try:
        lib = ctypes.CDLL(so_path)
        if hasattr(lib, "axon_start_nrt_profile"):
            lib.axon_start_nrt_profile.argtypes = [
                ctypes.POINTER(ctypes.c_int64), ctypes.c_size_t]
            lib.axon_start_nrt_profile.restype = ctypes.c_int64
            lib.axon_stop_nrt_profile.argtypes = [ctypes.c_char_p]
            lib.axon_stop_nrt_profile.restype = ctypes.c_int64

            @contextlib.contextmanager
            def hook(output_dir, device_ids):
                import jax
                jax.devices()
                if device_ids:
                    ids = (ctypes.c_int64 * len(device_ids))(*device_ids)
                    rc = lib.axon_start_nrt_profile(ids, len(device_ids))
                else:
                    rc = lib.axon_start_nrt_profile(None, 0)
                if rc != 0:
                    raise RuntimeError(f"axon_start_nrt_profile rc={rc}")
                try:
                    yield
                finally:
                    n = lib.axon_stop_nrt_profile(str(output_dir).encode())
                    print(f"profile: {n} file(s) written to {output_dir}")
    except OSError:
        pass
    mod = types.ModuleType("antenv.axon_hooks")
    _h = hook
    mod.get_axon_ntff_profile_hook = lambda: _h
    mod.set_axon_ntff_profile_hook = lambda h: None
    sys.modules["antenv.axon_hooks"] = mod


def kernel_traced(**inputs):
    _install_ntff_hook()
    from concourse import bass_utils
    bass_utils.upload_artifacts = lambda tmpdir: tmpdir  # no bucket in this sandbox
    try:
        return _prep_and_run(inputs, trace=True)
    except Exception as e:
        print("trace path failed (%s); rerunning untraced" % e)
        return _prep_and_run(inputs, trace=False)


# revision 38
# speedup vs baseline: 1.1620x; 1.1620x over previous
"""Trainium2 Bass kernel for the DCGSC SNN (delayed-current adaptive-LIF net).

Math (per layer, after folding BN + (1-alpha) into the weights):
    v_t = alpha * w_{t-1} + Itil_t - y_{t-1}
    w_t = v_t * 1[v_t <= TH]          (soft-reset membrane)
    s_t = 1[v_t > TH]                 (spike)
    y_t = rho * y_{t-1} + gamma * s_t (gamma = (1-alpha)*beta_a; y = (1-alpha)*a)
Itil_t comes from a PSUM matmul accumulation; the -y_{t-1} term is folded into
the same PSUM bank via two identity matmuls of the expanded adaptation:
    -y_{t-1} = (-rho*I) @ Y_{t-2} + (-I) @ sigma_{t-1},   sigma = gamma*s,
so the only chain-critical PE op after sigma_{t-1} is one FD=256 matmul.
Per-channel learnable delays are applied as time-offset DMA reads: layer-1
from a host-transposed zero-padded input, layer-2 via a DRAM spike staging
buffer read back with per-delay-group offsets (channels host-sorted by delay).
Output layer: mean_t(leaky v_out) == Wout @ (sum_t c_t * s2_t), accumulated
per-step in PSUM with c_t-scaled spikes.

The spike dynamics are chaotically sensitive (near-threshold flips cascade),
so every term feeding the membrane must be fp32-exact. Feedforward matmuls
use a scaled relative fp16 split: moving tensors (0/1 spikes, ones) carry a
global 2^-11 factor (exact in fp16), weights are hi*2^11 (exact exponent
shift) plus lo' = (W - hi)*2^11 — two fp16 matmuls reconstruct W@x to ~22
mantissa bits at 1 PE cycle/row. The adaptation state Y' = y/(gamma*2^11)
is updated directly from the staged spikes; its -rho*y matmul stays fp32.
Membrane v/w and Y' stay fp32 on the Vector engine.

Sharding: pure data-parallel, batch 512 -> 64 per core across 8 cores.
"""

import sys

sys.path.insert(0, "/opt/trn_rl_repo")

import numpy as np

B, T, FIN, H, C = 512, 100, 140, 512, 35
MAX_DELAY = 60
TH = 0.3
EPS = 1e-5
NCORES = 8
BL = B // NCORES  # 64
TB = 4  # time-block: one [128, 2*TB*128] PSUM tile (2 banks) per block
TPAD = T + MAX_DELAY


def _sigmoid32(x):
    x = np.asarray(x, np.float32)
    return (1.0 / (1.0 + np.exp(-x.astype(np.float64)))).astype(np.float32)


def _delays(delay_raw):
    return np.round(_sigmoid32(delay_raw) * np.float32(MAX_DELAY)).astype(np.int64)


def _groups(ds):
    """Runs of equal delay in sorted order, split at 128-partition chunks.
    Returns list of (chunk, p0, p1, delay)."""
    out = []
    i = 0
    n = len(ds)
    while i < n:
        j = i
        while j < n and ds[j] == ds[i]:
            j += 1
        s = i
        while s < j:
            e = min(j, (s // 128 + 1) * 128)
            out.append((s // 128, s % 128, (e - 1) % 128 + 1, int(ds[i])))
            s = e
        i = j
    return out


def _numpy_reference(i):
    """Fallback path (only used if params are not per-channel-constant)."""
    x = i["x"]

    def ad(x, draw):
        d = _delays(draw)
        Bb, Tt, Ff = x.shape
        xp = np.pad(x, ((0, 0), (MAX_DELAY, 0), (0, 0)))
        idx = np.arange(Tt)[:, None] + MAX_DELAY - d[None, :]
        return np.take_along_axis(xp, np.broadcast_to(idx[None], (Bb, Tt, Ff)), axis=1)

    def bn(v, g, b, m, s):
        return (v - m) / np.sqrt(s + EPS) * g + b

    def adlif(I, al, rh, ba):
        v = np.zeros(I.shape[1:], np.float32)
        a = np.zeros_like(v)
        s = np.zeros_like(v)
        out = []
        for t in range(I.shape[0]):
            v = al * v * (1 - s) + (1 - al) * (I[t] - a)
            s = (v > TH).astype(np.float32)
            a = rh * a + ba * s
            out.append(s)
        return np.stack(out)

    xd = ad(x, i["delay_raw1"])
    I1 = bn(np.einsum("btf,hf->bth", xd, i["W1"]), i["gamma1"], i["bias1"], i["mean1"], i["var1"])
    s1 = adlif(np.transpose(I1, (1, 0, 2)), i["alpha1"], i["rho1"], i["beta_a1"])
    sd = ad(np.transpose(s1, (1, 0, 2)), i["delay_raw2"])
    I2 = bn(np.einsum("bth,gh->btg", sd, i["W2"]), i["gamma2"], i["bias2"], i["mean2"], i["var2"])
    s2 = adlif(np.transpose(I2, (1, 0, 2)), i["alpha2"], i["rho2"], i["beta_a2"])
    Io = np.einsum("tbh,ch->tbc", s2, i["Wout"])
    v = np.zeros(Io.shape[1:], np.float32)
    acc = np.zeros_like(v)
    for t in range(T):
        v = i["beta_out"] * v + (1 - i["beta_out"]) * Io[t]
        acc += v
    return (acc / T).astype(np.float32)


def _build_program(d1groups, d2groups, sc):
    """Emit the SPMD Bass/Tile program (identical across cores)."""
    import concourse.bacc as bacc
    import concourse.mybir as mybir
    import concourse.tile as tile
    from contextlib import ExitStack

    f32 = mybir.dt.float32
    bf16 = mybir.dt.float16  # 11-bit mantissa: split hi+lo covers 22+ bits
    AL = mybir.AluOpType

    def R(ap):
        return ap

    nc = bacc.Bacc(
        "TRN2",
        target_bir_lowering=False,
        debug=False,
        enable_asserts=False,
        num_devices=NCORES,
    )

    xt_d = nc.dram_tensor("xt", [FIN, T, BL], bf16, kind="ExternalInput")
    w1ah_d = nc.dram_tensor("w1ah", [128, H], bf16, kind="ExternalInput")
    w1al_d = nc.dram_tensor("w1al", [128, H], bf16, kind="ExternalInput")
    w1b_d = nc.dram_tensor("w1b", [26, H], bf16, kind="ExternalInput")
    w2h_d = nc.dram_tensor("w2h", [H, H], bf16, kind="ExternalInput")
    w2l_d = nc.dram_tensor("w2l", [H, H], bf16, kind="ExternalInput")
    c2_d = nc.dram_tensor("c2", [2, H], bf16, kind="ExternalInput")
    wo_d = nc.dram_tensor("wo", [H, C], f32, kind="ExternalInput")
    ones_d = nc.dram_tensor("onesr", [2, T * BL], bf16, kind="ExternalInput")
    idn_d = nc.dram_tensor("idn", [128, 128], bf16, kind="ExternalInput")
    ngh1_d = nc.dram_tensor("ngh1", [128, 128], bf16, kind="ExternalInput")
    ngl1_d = nc.dram_tensor("ngl1", [128, 128], bf16, kind="ExternalInput")
    ngh2_d = nc.dram_tensor("ngh2", [128, 128], bf16, kind="ExternalInput")
    ngl2_d = nc.dram_tensor("ngl2", [128, 128], bf16, kind="ExternalInput")
    nr1_d = nc.dram_tensor("nr1", [128, 128], f32, kind="ExternalInput")
    nr2_d = nc.dram_tensor("nr2", [128, 128], f32, kind="ExternalInput")
    out_d = nc.dram_tensor("out", [BL, C], f32, kind="ExternalOutput")
    import os as _os
    _dbg = _os.environ.get("BASSDBG") == "1"
    sdram = nc.dram_tensor("sdram", [H, T, BL], bf16,
                           kind="ExternalOutput" if _dbg else "Internal")

    with tile.TileContext(nc) as tc:
        with ExitStack() as ctx:
            P = ctx.enter_context(tc.tile_pool(name="persist", bufs=1))
            PS = ctx.enter_context(tc.tile_pool(name="pstep", bufs=3, space="PSUM"))
            PX = ctx.enter_context(tc.tile_pool(name="psmall", bufs=1, space="PSUM"))
            VP = ctx.enter_context(tc.tile_pool(name="vp", bufs=6))
            SP = ctx.enter_context(tc.tile_pool(name="sp", bufs=8))

            xda = P.tile([128, T, BL], bf16, name="xda", tag="xda")
            xdb = P.tile([26, T, BL], bf16, name="xdb", tag="xdb")
            sdt = [P.tile([128, T, BL], bf16, name=f"sdt{k}", tag=f"sdt{k}") for k in range(4)]
            w1ah = P.tile([128, H], bf16, name="w1ah", tag="w1ah")
            w1al = P.tile([128, H], bf16, name="w1al", tag="w1al")
            w1b = P.tile([26, H], bf16, name="w1b", tag="w1b")
            w2h = [P.tile([128, H], bf16, name=f"w2h{k}", tag=f"w2h{k}") for k in range(4)]
            w2l = [P.tile([128, H], bf16, name=f"w2l{k}", tag=f"w2l{k}") for k in range(4)]
            c2r = P.tile([2, H], bf16, name="c2r", tag="c2r")
            wo = [P.tile([128, C], f32, name=f"wo{k}", tag=f"wo{k}") for k in range(4)]
            idn = P.tile([128, 128], bf16, name="idn", tag="idn")
            ngh = [P.tile([128, 128], bf16, name=f"ngh{l}", tag=f"ngh{l}") for l in range(2)]
            ngl = [P.tile([128, 128], bf16, name=f"ngl{l}", tag=f"ngl{l}") for l in range(2)]
            nr1 = P.tile([128, 128], f32, name="nr1", tag="nr1")
            nr2 = P.tile([128, 128], f32, name="nr2", tag="nr2")
            w1t = P.tile([128, 256], f32, name="w1t", tag="w1t")
            w2t = P.tile([128, 256], f32, name="w2t", tag="w2t")
            Y1 = [P.tile([128, 256], f32, name=f"Y1{k}", tag=f"Y1{k}") for k in range(2)]
            Y2 = [P.tile([128, 256], f32, name=f"Y2{k}", tag=f"Y2{k}") for k in range(2)]
            ones1 = P.tile([2, TB * BL], bf16, name="ones1", tag="ones1")
            Ssb = P.tile([128, 256], f32, name="Ssb", tag="Ssb")
            osb = P.tile([BL, C], f32, name="osb", tag="osb")

            # weight loads
            nc.scalar.dma_start(w1ah[:], w1ah_d.ap())
            nc.scalar.dma_start(w1al[:], w1al_d.ap())
            nc.scalar.dma_start(w1b[:], w1b_d.ap())
            nc.scalar.dma_start(ngh[0][:], ngh1_d.ap())
            nc.scalar.dma_start(ngl[0][:], ngl1_d.ap())
            nc.scalar.dma_start(nr1[:], nr1_d.ap())

            def load_l2_weights():
                w2hr = w2h_d.ap().rearrange("(k p) h -> k p h", p=128)
                w2lr = w2l_d.ap().rearrange("(k p) h -> k p h", p=128)
                for k in range(4):
                    nc.scalar.dma_start(w2h[k][:], w2hr[k])
                    nc.sync.dma_start(w2l[k][:], w2lr[k])
                nc.sync.dma_start(c2r[:], c2_d.ap())
                wor = wo_d.ap().rearrange("(k p) c -> k p c", p=128)
                for k in range(4):
                    nc.scalar.dma_start(wo[k][:], wor[k])
                nc.sync.dma_start(idn[:], idn_d.ap())
                nc.scalar.dma_start(ngh[1][:], ngh2_d.ap())
                nc.scalar.dma_start(ngl[1][:], ngl2_d.ap())
                nc.sync.dma_start(nr2[:], nr2_d.ap())

            # zero-init (layer-2 delayed-spike pads + scan state)
            for k in range(4):
                nc.gpsimd.memset(sdt[k][:], 0.0)
            for tl in (w1t, w2t):
                nc.vector.memset(tl[:], 0.0)
            for tl in (Y1[0], Y1[1], Y2[0], Y2[1]):
                nc.vector.memset(tl[:], 0.0)
            nc.vector.memset(ones1[:], 1.0 / 2048.0)

            # layer-1 delays are applied host-side; xt is already shifted and
            # zero-padded, so the input loads are dense full-partition DMAs.
            # Tail channels (128:140) load twice: rows 0-11 for the hi
            # weights, rows 12-23 for the lo weights.
            _dmae = [nc.sync, nc.scalar]
            nc.sync.dma_start(xda[:, :, :], xt_d.ap()[0:128, :, :])
            nc.sync.dma_start(xdb[0:12, :, :], xt_d.ap()[128:140, :, :])
            nc.sync.dma_start(xdb[12:24, :, :], xt_d.ap()[128:140, :, :])
            # bias rows (ones, hi+lo)
            nc.sync.dma_start(xdb[24:26].rearrange("p t b -> p (t b)"), ones_d.ap())

            sdram_r = sdram.ap().rearrange("(k p) t b -> k p t b", p=128)

            stage_at = {}

            def scan(layer, after_block=None):
                wt = w1t if layer == 1 else w2t
                Ys = Y1 if layer == 1 else Y2
                nrho = nr1 if layer == 1 else nr2
                alpha = sc["a1"] if layer == 1 else sc["a2"]
                rho = sc["r1"] if layer == 1 else sc["r2"]
                th = sc["th1"] if layer == 1 else sc["th2"]
                nh, nl = ngh[layer - 1], ngl[layer - 1]
                sig_prev = None
                psS = None
                if layer == 2:
                    psS = PX.tile([128, 256], f32, name="psS", tag="psS")
                for blk in range(T // TB):
                    t0 = blk * TB
                    ps = PS.tile([128, 2 * TB * 128], f32, name="ps", tag="ps")
                    # column map: bq*TB*128 + t*128 + qi*64 + b  (q = 2*bq+qi)
                    psv = ps[:].rearrange("p (bq t qi b) -> p bq t qi b",
                                          bq=2, t=TB, qi=2)
                    # NOTE: start=True clears has_written for the WHOLE 2KB
                    # bank, so only the first matmul into each bank carries it.
                    for q in range(4):
                        bq, qi = q // 2, q % 2
                        dst = psv[:, bq, :, qi, :]
                        if layer == 1:
                            nc.tensor.matmul(
                                dst, w1ah[:, q * 128 : (q + 1) * 128],
                                xda[:, t0 : t0 + TB, :],
                                start=(qi == 0), stop=False, skip_group_check=True)
                            nc.tensor.matmul(
                                dst, w1al[:, q * 128 : (q + 1) * 128],
                                xda[:, t0 : t0 + TB, :],
                                start=False, stop=False, skip_group_check=True)
                            nc.tensor.matmul(
                                dst, w1b[:, q * 128 : (q + 1) * 128],
                                xdb[:, t0 : t0 + TB, :],
                                start=False, stop=False, skip_group_check=True)
                        else:
                            for k in range(4):
                                nc.tensor.matmul(
                                    dst, w2h[k][:, q * 128 : (q + 1) * 128],
                                    sdt[k][:, t0 : t0 + TB, :],
                                    start=(qi == 0 and k == 0), stop=False,
                                    skip_group_check=True)
                                nc.tensor.matmul(
                                    dst, w2l[k][:, q * 128 : (q + 1) * 128],
                                    sdt[k][:, t0 : t0 + TB, :],
                                    start=False, stop=False, skip_group_check=True)
                            nc.tensor.matmul(
                                dst, c2r[:, q * 128 : (q + 1) * 128], ones1[:],
                                start=False, stop=False, skip_group_check=True)
                    for tt in range(TB):
                        t = t0 + tt
                        # -y_{t-1} = (-rho I) @ Y_{t-2}  +  (-I) @ sigma_{t-1}
                        for bq in range(2):
                            pb = ps[:, bq * TB * 128 + tt * 128 :
                                       bq * TB * 128 + (tt + 1) * 128]
                            if t >= 2:
                                nc.tensor.matmul(
                                    pb, nrho[:], Ys[t % 2][:, bq * 128 : (bq + 1) * 128],
                                    start=False, stop=False, skip_group_check=True)
                            if t >= 1:
                                nc.tensor.matmul(
                                    pb, nh[:], sig_prev[:, bq * 128 : (bq + 1) * 128],
                                    start=False, stop=False, skip_group_check=True)
                                nc.tensor.matmul(
                                    pb, nl[:], sig_prev[:, bq * 128 : (bq + 1) * 128],
                                    start=False, stop=(tt == TB - 1),
                                    skip_group_check=True)
                        pst = psv[:, :, tt, :, :]
                        v = VP.tile([128, 256], f32, name="v", tag="v")
                        vv = v[:].rearrange("p (bq qi b) -> p bq qi b", bq=2, qi=2)
                        wv = wt[:].rearrange("p (bq qi b) -> p bq qi b", bq=2, qi=2)
                        nc.vector.scalar_tensor_tensor(
                            vv, wv, alpha, pst, op0=AL.mult, op1=AL.add)
                        s01 = SP.tile([128, 256], bf16, name=f"s01{layer}", tag=f"s01{layer}")
                        nc.vector.tensor_scalar(
                            s01[:], v[:], th, float(sc["s01"]),
                            op0=AL.is_gt, op1=AL.mult)
                        nc.vector.scalar_tensor_tensor(
                            wt[:], v[:], th, v[:], op0=AL.is_le, op1=AL.mult)
                        if layer == 2:
                            sh = SP.tile([128, 256], bf16, name="sh", tag="sh")
                            nc.vector.tensor_scalar(
                                sh[:], v[:], th, float(sc["ct"][t]),
                                op0=AL.is_gt, op1=AL.mult)
                            nc.tensor.matmul(
                                psS[:], idn[:], sh[:], start=(t == 0), stop=(t == T - 1),
                                skip_group_check=True)
                        # y_t = rho*y_{t-1} + sigma_t (one step of slack before
                        # its consumer, the t+2 nrho matmul)
                        nc.vector.scalar_tensor_tensor(
                            Ys[t % 2][:], Ys[(t + 1) % 2][:], rho, s01[:],
                            op0=AL.mult, op1=AL.add)
                        if layer == 1:
                            stage_at[t] = nc.sync.dma_start(
                                sdram.ap()[:, t, :].rearrange("(k p) b -> p k b", p=128),
                                s01[:].rearrange("p (k b) -> p k b", b=64))
                        sig_prev = s01
                    if after_block and blk in after_block and tt == TB - 1:
                        after_block[blk]()
                return psS

            from concourse.bass import _add_dep_helper

            NCH = 5
            CH = T // NCH

            def emit_d2_chunk(j):
                # layer-2 delayed spikes: sdt[k][p,t,b] = sig1[k*128+p, t-d, b].
                # Emitted mid-L1-scan, gated on the staging DMA of the last
                # needed step via explicit dep edges (no barrier); qACT only,
                # so the reads never queue ahead of qSP staging writes.
                ta, tb_ = j * CH, (j + 1) * CH
                gate = stage_at[tb_ - 1]
                for ch, p0, p1, d in d2groups:
                    lo = max(ta, d)
                    if lo < tb_:
                        inst = nc.scalar.dma_start(
                            sdt[ch][p0:p1, lo:tb_, :],
                            sdram_r[ch, p0:p1, lo - d : tb_ - d, :])
                        _add_dep_helper(
                            getattr(inst, "ins", inst),
                            getattr(gate, "ins", gate),
                            sync=True, reason="sdram staging RAW")

            cbs = {0: load_l2_weights}
            for j in range(NCH):
                cbs[(CH * (j + 1) - 1) // TB] = (lambda jj: lambda: emit_d2_chunk(jj))(j)

            scan(1, after_block=cbs)

            psS = scan(2)

            # output: out[b, c] = sum_q S[q-chunk]^T @ WoutT[q-chunk]
            nc.vector.tensor_copy(Ssb[:], psS[:])
            psO = PX.tile([BL, C], f32, name="psO", tag="psO")
            for q in range(4):
                nc.tensor.matmul(
                    psO[:], Ssb[:, q * 64 : (q + 1) * 64], wo[q][:],
                    start=(q == 0), stop=(q == 3), skip_group_check=True)
            nc.vector.tensor_copy(osb[:], psO[:])
            nc.sync.dma_start(out_d.ap(), osb[:])

    nc.compile()
    return nc


_CACHE = {}


def _prep_and_run(inputs, trace=False):
    i = {k: np.asarray(v, np.float32) for k, v in inputs.items()}
    const = all(
        np.ptp(np.asarray(i[k], np.float64)) == 0.0
        for k in ("alpha1", "rho1", "beta_a1", "alpha2", "rho2", "beta_a2", "beta_out")
    )
    if not const or i["x"].shape != (B, T, FIN):
        return _numpy_reference(i), None

    d1 = _delays(i["delay_raw1"])
    d2 = _delays(i["delay_raw2"])
    fperm = np.argsort(d1, kind="stable")
    hperm = np.argsort(d2, kind="stable")
    d1groups = _groups(d1[fperm])
    d2groups = _groups(d2[hperm])

    a1 = float(i["alpha1"][0]); r1 = float(i["rho1"][0]); b1 = float(i["beta_a1"][0])
    a2 = float(i["alpha2"][0]); r2 = float(i["rho2"][0]); b2 = float(i["beta_a2"][0])
    bo = float(i["beta_out"][0])
    ga1 = np.float32((1 - a1) * b1)
    ga2 = np.float32((1 - a2) * b2)

    bf = np.float16

    # no rescale: sigma stays fp32 (exact gamma); layer-2 input spikes are
    # staged raw as 0/1 (exact in fp16), with gamma folded into nothing.
    gt1 = float(ga1)
    gt2 = float(ga2)
    th1 = TH
    th2 = TH

    S11 = np.float64(2048.0)  # moving tensors carry 2^-11; weights carry 2^11

    def split_bf(a):
        # hi*2^11 is an exact fp16 exponent shift; lo' = (a - hi)*2^11 recovers
        # ~11 more mantissa bits. Paired with 2^-11-scaled spikes the products
        # reconstruct a*s to ~2^-22 relative - fp32-class, flip-free.
        a64 = np.asarray(a, np.float64)
        hi = np.asarray(a, np.float32).astype(bf)
        hi2 = (hi.astype(np.float32) * np.float32(S11)).astype(bf)
        lo2 = ((a64 - hi.astype(np.float64)) * S11).astype(np.float32).astype(bf)
        return hi2, lo2

    g1 = (i["gamma1"].astype(np.float64) / np.sqrt(i["var1"].astype(np.float64) + EPS))
    W1f = (i["W1"].astype(np.float64) * g1[:, None] * (1 - a1))[hperm][:, fperm]
    c1f = ((i["bias1"].astype(np.float64) - i["mean1"].astype(np.float64) * g1)[hperm]
           * (1 - a1))
    g2 = (i["gamma2"].astype(np.float64) / np.sqrt(i["var2"].astype(np.float64) + EPS))
    W2f = (i["W2"].astype(np.float64) * g2[:, None] * (1 - a2))[:, hperm]
    c2f = (i["bias2"].astype(np.float64) - i["mean2"].astype(np.float64) * g2) * (1 - a2)

    w1ah, w1al = split_bf(np.ascontiguousarray(W1f[:, :128].T))      # [128, 512]
    w1b = np.zeros((26, H), np.float64)
    tl_hi, tl_lo = split_bf(W1f[:, 128:].T)                          # [12, 512]
    c1_hi, c1_lo = split_bf(c1f)
    w1b = np.zeros((26, H), bf)
    w1b[:12] = tl_hi; w1b[12:24] = tl_lo; w1b[24] = c1_hi; w1b[25] = c1_lo
    w2h, w2l = split_bf(np.ascontiguousarray(W2f.T))                 # [512h1, 512g]
    c2_hi, c2_lo = split_bf(c2f)
    c2r = np.stack([c2_hi, c2_lo])                                   # [2, 512]
    woT = np.ascontiguousarray(i["Wout"].T, np.float32)              # [512, 35]
    ct = ((1.0 - bo ** (T - np.arange(T))) / T).astype(np.float32)

    eye = np.eye(128, dtype=np.float32)
    # -y matmul weights: A = gamma*2^11 paired with s*2^-11 spikes; hi/lo
    # split recovers gamma to ~22 bits. Y' tracks y/(gamma*2^11) so its
    # update input is the raw s*2^-11 tensor.
    def gsplit(g):
        A = np.float64(g) * 2048.0
        hi = np.float16(A)
        lo = np.float16(np.float32(A - np.float64(hi)))
        return float(hi), float(lo)
    g1h, g1l = gsplit(ga1)
    g2h, g2l = gsplit(ga2)
    sc = dict(a1=a1, r1=r1, a2=a2, r2=r2, gt1=gt1, gt2=gt2,
              th1=th1, th2=th2, ct=ct, s01=1.0 / 2048.0)

    key = (tuple(d1groups), tuple(d2groups),
           a1, r1, b1, a2, r2, b2, bo)
    if key not in _CACHE:
        _CACHE[key] = _build_program(d1groups, d2groups, sc)
    nc = _CACHE[key]

    # host-applied layer-1 delays: xt[f, t, b] = x[f, t - d_f, b] (zero pad),
    # at the 2^-11 moving-tensor scale
    xsrc = (i["x"].transpose(2, 1, 0)[fperm] * np.float32(1.0 / 2048.0)).astype(bf)
    xt_full = np.zeros((FIN, T, B), bf)
    ds = d1[fperm]
    for f in range(FIN):
        d = int(ds[f])
        if d < T:
            xt_full[f, d:T] = xsrc[f, 0 : T - d]
    shared = dict(w1ah=w1ah, w1al=w1al, w1b=w1b, w2h=w2h, w2l=w2l, c2=c2r, wo=woT,
                  onesr=np.full((2, T * BL), 1.0 / 2048.0, bf),
                  idn=eye.astype(bf),
                  ngh1=(-np.float32(g1h) * eye).astype(bf),
                  ngl1=(-np.float32(g1l) * eye).astype(bf),
                  ngh2=(-np.float32(g2h) * eye).astype(bf),
                  ngl2=(-np.float32(g2l) * eye).astype(bf),
                  nr1=np.float32(-r1 * float(ga1) * 2048.0) * eye,
                  nr2=np.float32(-r2 * float(ga2) * 2048.0) * eye)
    in_maps = []
    for c in range(NCORES):
        m = dict(shared)
        m["xt"] = np.ascontiguousarray(xt_full[:, :, c * BL : (c + 1) * BL])
        in_maps.append(m)

    from concourse.bass_utils import run_bass_kernel_spmd

    res = run_bass_kernel_spmd(nc, in_maps, list(range(NCORES)), trace=trace)
    out = np.concatenate([res.results[c]["out"] for c in range(NCORES)], axis=0)
    return out.astype(np.float32), res


def kernel(**inputs):
    out, _ = _prep_and_run(inputs, trace=False)
    return out


def _install_ntff_hook():
    """Provide antenv.axon_hooks (missing in this image) so trace=True works."""
    import types, ctypes, contextlib

    try:
        import antenv.axon_hooks  # noqa: F401
        return
    except ImportError:
        pass
    so_path = "/opt/axon/libaxon_pjrt.so"
    hook = None
    try:
        lib = ctypes.CDLL(so_path)
        if hasattr(lib, "axon_start_nrt_profile"):
            lib.axon_start_nrt_profile.argtypes = [
                ctypes.POINTER(ctypes.c_int64), ctypes.c_size_t]
            lib.axon_start_nrt_profile.restype = ctypes.c_int64
            lib.axon_stop_nrt_profile.argtypes = [ctypes.c_char_p]
            lib.axon_stop_nrt_profile.restype = ctypes.c_int64

            @contextlib.contextmanager
            def hook(output_dir, device_ids):
                import jax
                jax.devices()
                if device_ids:
                    ids = (ctypes.c_int64 * len(device_ids))(*device_ids)
                    rc = lib.axon_start_nrt_profile(ids, len(device_ids))
                else:
                    rc = lib.axon_start_nrt_profile(None, 0)
                if rc != 0:
                    raise RuntimeError(f"axon_start_nrt_profile rc={rc}")
                try:
                    yield
                finally:
                    n = lib.axon_stop_nrt_profile(str(output_dir).encode())
                    print(f"profile: {n} file(s) written to {output_dir}")
    except OSError:
        pass
    mod = types.ModuleType("antenv.axon_hooks")
    _h = hook
    mod.get_axon_ntff_profile_hook = lambda: _h
    mod.set_axon_ntff_profile_hook = lambda h: None
    sys.modules["antenv.axon_hooks"] = mod


def kernel_traced(**inputs):
    _install_ntff_hook()
    from concourse import bass_utils
    bass_utils.upload_artifacts = lambda tmpdir: tmpdir  # no bucket in this sandbox
    try:
        return _prep_and_run(inputs, trace=True)
    except Exception as e:
        print("trace path failed (%s); rerunning untraced" % e)
        return _prep_and_run(inputs, trace=False)
